# revision 2
# baseline (speedup 1.0000x reference)
"""Multi-head attention (b=4, n=2048, dim=1024, heads=16) on 8 TRN2 cores — v8.

Sharding: tensor-parallel over heads (2 heads per core) + row-parallel output
projection; host sums the 8 partial outputs and adds the bias.

Hardware model (measured on hw):
  - matmul wall ~= N_free * 0.45ns regardless of K/M (K<=64 & M<=64 is 2x slow;
    same-bank bf16 accumulation needs >=3-call spacing; PE clock needs ~3us of
    gap-free execution to reach 2.4GHz, so every PE stall costs extra).
  - ACT exp ~1146ns per [128,1024] tile; 256 tiles/core = ~286us — the
    attention-phase floor. ACT does nothing else during attention.
  - fp8 anywhere in the q/k/v/score path fails the 2e-2 gate (softmax weight
    noise passes ~1:1 to the output), so all matmul data paths are bf16.

Phase 1 (16 blocks of 512 tokens): QKV projection, all bf16.
  q/k/v chains rotate 3 psum banks (hazard-free), all double-buffered;
  x streamed contiguously ([p, blk, kt, i] host layout); K/V drains on ACT
  (idle here), Q on DVE. v transposed via PE into Vt [128, g, 2, 65]
  (per-head [v(64) | ones]); the ones column gives the softmax denominator.

Phase 2 (16 chunks of 512 tokens): attention + out-projection.
  PSUM: st 2x[128,2,512] scores (both heads) | po0/po1 [65,512] AV accum |
  pp 2x[128,512] out-proj = 8 banks.
  Global pipeline over (chunk, jtile): scores+exp lead, AV lags 6 steps so
  chunk-boundary drains never block the score/exp stream. exp [128,1024] on
  ACT -> et bf16 (12 buffers).
  Chunk end: po drained with two [65,512] f32->bf16 DVE copies (o rows +
  denominator row together); o_sb rebuilt via Pool copy (h0) + SBUF DMA (h1);
  denominators (bf16) to DRAM, repacked [128,8], DVE reciprocal (f32),
  partition-broadcast DMA, Pool multiply — all scheduled into later steps so
  neither DVE nor PE head-of-line blocks. Out-proj (8 matmuls/chunk into pp,
  DVE cast, DMA out bf16) fills early steps (chunks >=2 old) and late steps
  (>=1 old). Host sums partials + bias.
"""

import os
import sys
import types

import numpy as np

if "antenv.axon_hooks" not in sys.modules:
    try:
        from trn_agent_boot.trn_boot import _ntff_profile_via_ctypes

        _m = types.ModuleType("antenv.axon_hooks")
        _h = _ntff_profile_via_ctypes("/opt/axon/libaxon_pjrt.so")
        _m.get_axon_ntff_profile_hook = lambda: _h
        _m.set_axon_ntff_profile_hook = lambda hook: None
        sys.modules["antenv.axon_hooks"] = _m
    except Exception:
        pass

import ml_dtypes

import concourse.bacc as bacc
import concourse.bass as bass
import concourse.mybir as mybir
import concourse.tile as tile
from concourse.bass_utils import run_bass_kernel_spmd
from concourse.masks import make_identity

F32 = mybir.dt.float32
BF16 = mybir.dt.bfloat16

B, N, DIM, HEADS = 4, 2048, 1024, 16
HD = DIM // HEADS          # 64
NCORES = 8
HPC = HEADS // NCORES      # 2
NT = B * N                 # 8192
SCALE = HD ** -0.5         # 0.125

NBLK = NT // 512           # 16
NCH = NT // 512            # 16
JT = N // 128              # 16
GJT = B * JT               # 64
LAG = 6


def _build_nc():
    nc = bacc.Bacc("TRN2", target_bir_lowering=False, debug=False)

    xTb = nc.dram_tensor("xTb", [128, NBLK, 8, 512], BF16, kind="ExternalInput")
    wqk = nc.dram_tensor("wqk", [128, 8, 2, 128], BF16, kind="ExternalInput")
    wv = nc.dram_tensor("wv", [128, 8, 128], BF16, kind="ExternalInput")
    wout = nc.dram_tensor("wout", [128, DIM], BF16, kind="ExternalInput")
    po = nc.dram_tensor("po", [DIM, NT], BF16, kind="ExternalOutput")
    dn_dram = nc.dram_tensor("dn_dram", [NCH, 1024], F32)
    rc_dram = nc.dram_tensor("rc_dram", [NCH, 1024], F32)

    with tile.TileContext(nc) as tc:
        with (
            tc.tile_pool(name="big", bufs=1) as big,
            tc.tile_pool(name="strm", bufs=2) as strm,
            tc.tile_pool(name="etp", bufs=12) as etp,
        ):
            # ---- persistent SBUF ----
            QT = big.tile([128, NT], BF16)
            KT = big.tile([128, GJT, 128], BF16)
            Vt = big.tile([128, GJT, 2, 65], BF16)
            o_sb = big.tile([128, NT], BF16)
            wqk_sb = big.tile([128, 8, 2, 128], BF16)
            wv_sb = big.tile([128, 8, 128], BF16)
            wout_sb = big.tile([128, DIM], BF16)
            ident = big.tile([128, 128], BF16)

            # critical-path first: qk/v weights, then block-0 x (split so the
            # first k-tiles land fast); wout deferred.
            nc.sync.dma_start(out=wqk_sb, in_=wqk[:, :, :, :])
            nc.sync.dma_start(out=wv_sb, in_=wv[:, :, :])

            # ================= Phase 1: QKV projection =================
            with tc.tile_pool(name="ps1", bufs=1, space="PSUM") as ps1:
                for blk in range(NBLK):
                    ncol = slice(blk * 512, (blk + 1) * 512)
                    xin = strm.tile([128, 8, 512], BF16, tag="xin", bufs=3)
                    if blk == 0:
                        nc.sync.dma_start(
                            out=xin[:, 0:2, :], in_=xTb[:, 0, 0:2, :]
                        )
                        nc.sync.dma_start(
                            out=xin[:, 2:8, :], in_=xTb[:, 0, 2:8, :]
                        )
                        make_identity(nc, ident)
                        nc.vector.memset(Vt[:, :, :, 64], 1.0)
                    else:
                        nc.sync.dma_start(out=xin, in_=xTb[:, blk, :, :])
                    if blk == 1:
                        nc.sync.dma_start(out=wout_sb, in_=wout[:, :])
                    pq = ps1.tile([128, 512], F32, tag="pq", bufs=2,
                                  name=f"pq{blk}")
                    pk = ps1.tile([128, 512], F32, tag="pk", bufs=2,
                                  name=f"pk{blk}")
                    pv = ps1.tile([128, 512], F32, tag="pv", bufs=2,
                                  name=f"pv{blk}")
                    for kt in range(8):
                        nc.tensor.matmul(
                            pq, wqk_sb[:, kt, 0, :], xin[:, kt, :],
                            start=(kt == 0), stop=(kt == 7),
                        )
                        nc.tensor.matmul(
                            pk, wqk_sb[:, kt, 1, :], xin[:, kt, :],
                            start=(kt == 0), stop=(kt == 7),
                        )
                        nc.tensor.matmul(
                            pv, wv_sb[:, kt, :], xin[:, kt, :],
                            start=(kt == 0), stop=(kt == 7),
                        )
                    # drains: DVE for Q, ACT (idle in phase 1) for K/v/Vt
                    nc.vector.tensor_copy(QT[:, ncol], pq)
                    nc.scalar.copy(
                        KT[:, blk * 4:(blk + 1) * 4, :],
                        pk.rearrange("p (g j) -> p g j", j=128),
                    )
                    vstage = strm.tile([128, 512], BF16, tag="vst")
                    nc.scalar.copy(vstage, pv)
                    tp = ps1.tile(
                        [128, 4, 128], BF16, tag="tp", bufs=2, name=f"tp{blk}"
                    )
                    for c4 in range(4):
                        nc.tensor.transpose(
                            tp[:, c4, :], vstage[:, c4 * 128:(c4 + 1) * 128],
                            ident,
                        )
                    g0 = blk * 4
                    for c4 in range(4):
                        nc.scalar.copy(
                            Vt[:, g0 + c4, :, 0:64],
                            tp[:, c4, :].rearrange("p (h f) -> p h f", h=2),
                        )

            # ================= Phase 2: attention + out-proj ===========
            with tc.tile_pool(name="ps2", bufs=1, space="PSUM") as ps2:
                proj_q = []      # (ch, mt) pending out-proj work
                fin_q = []       # chunks with pending normalize finisher

                def emit_proj_one(tail=False):
                    ch, mt = proj_q.pop(0)
                    icols = slice(ch * 512, (ch + 1) * 512)
                    pp = ps2.tile(
                        [128, 512], F32, tag="pp", bufs=2, name=f"pp{ch}_{mt}"
                    )
                    nc.tensor.matmul(
                        pp, wout_sb[:, mt * 128:(mt + 1) * 128],
                        o_sb[:, icols], start=True, stop=True,
                    )
                    pout = strm.tile([128, 512], BF16, tag="pout", bufs=4)
                    if tail and mt % 2 == 0:
                        nc.scalar.copy(pout, pp)
                    else:
                        nc.vector.tensor_copy(pout, pp)
                    nc.sync.dma_start(
                        out=po[mt * 128:(mt + 1) * 128, icols], in_=pout
                    )

                def emit_finisher():
                    ch = fin_q.pop(0)
                    icols = slice(ch * 512, (ch + 1) * 512)
                    pack = strm.tile([128, 8], F32, tag="pack")
                    nc.gpsimd.dma_start(
                        out=pack,
                        in_=dn_dram[ch:ch + 1, :].rearrange(
                            "o (p f) -> (o p) f", p=128
                        ),
                    )
                    with nc.allow_low_precision(reason="softmax denom recip"):
                        nc.vector.reciprocal(pack, pack)
                    nc.gpsimd.dma_start(
                        out=rc_dram[ch:ch + 1, :].rearrange(
                            "o (p f) -> (o p) f", p=128
                        ),
                        in_=pack,
                    )
                    bcast = strm.tile([128, 512], F32, tag="bcast")
                    for h in range(2):
                        src = rc_dram[ch:ch + 1, h * 512:(h + 1) * 512]
                        rbc = bass.AP(
                            tensor=src.tensor,
                            offset=src.offset,
                            ap=[[0, 64]] + list(src.ap)[1:],
                        )
                        nc.gpsimd.dma_start(
                            out=bcast[h * 64:(h + 1) * 64, :], in_=rbc
                        )
                    # normalize multiply on Pool (SBUF-only)
                    nc.gpsimd.tensor_mul(o_sb[:, icols], o_sb[:, icols], bcast)
                    for mt in range(DIM // 128):
                        proj_q.append((ch, mt))

                seq = [(c_, j_) for c_ in range(NCH) for j_ in range(JT)]
                po_tiles = {}
                ets = {}
                for idx in range(len(seq) + LAG):
                    if idx < len(seq):
                        ch, jt = seq[idx]
                        b = ch // 4
                        icols = slice(ch * 512, (ch + 1) * 512)
                        if jt == 0:
                            po_tiles[ch] = (
                                ps2.tile([65, 512], F32, tag="po0",
                                         name=f"po0_{ch}"),
                                ps2.tile([65, 512], F32, tag="po1",
                                         name=f"po1_{ch}"),
                            )
                        g = b * JT + jt
                        stt = ps2.tile(
                            [128, 2, 512], F32, tag="st", bufs=2,
                            name=f"st{ch}_{jt}",
                        )
                        nc.tensor.matmul(
                            stt[:, 0, :], KT[0:64, g, :], QT[0:64, icols],
                            start=True, stop=True,
                        )
                        nc.tensor.matmul(
                            stt[:, 1, :], KT[64:128, g, :],
                            QT[64:128, icols], start=True, stop=True,
                        )
                        et = etp.tile([128, 2, 512], BF16, tag="et", name="et")
                        nc.scalar.activation(
                            et.rearrange("p a b -> p (a b)"),
                            stt.rearrange("p a b -> p (a b)"),
                            mybir.ActivationFunctionType.Exp,
                            scale=SCALE,
                        )
                        ets[(ch, jt)] = et
                    if idx >= LAG:
                        ch2, jp = seq[idx - LAG]
                        b2 = ch2 // 4
                        g2 = b2 * JT + jp
                        et2 = ets.pop((ch2, jp))
                        p0, p1 = po_tiles[ch2]
                        nc.tensor.matmul(
                            p0, Vt[:, g2, 0, :], et2[:, 0, :],
                            start=(jp == 0), stop=(jp == JT - 1),
                        )
                        nc.tensor.matmul(
                            p1, Vt[:, g2, 1, :], et2[:, 1, :],
                            start=(jp == 0), stop=(jp == JT - 1),
                        )
                        if jp == JT - 1:
                            # drain AV psum: one [65,512] f32->bf16 copy per
                            # head frees the bank fast; o_sb rebuilt off the
                            # critical path (Pool copy / SBUF DMA).
                            ic2 = slice(ch2 * 512, (ch2 + 1) * 512)
                            nc.vector.tensor_copy(
                                o_sb[0:64, ic2], p0[0:64, :]
                            )
                            h1s = strm.tile([64, 512], BF16, tag="h1s")
                            nc.vector.tensor_copy(h1s, p1[0:64, :])
                            nc.sync.dma_start(out=o_sb[64:128, ic2], in_=h1s)
                            dnst = strm.tile([1, 1024], F32, tag="dnst")
                            nc.vector.tensor_copy(
                                dnst[0:1, 0:512], p0[64:65, :]
                            )
                            nc.vector.tensor_copy(
                                dnst[0:1, 512:1024], p1[64:65, :]
                            )
                            nc.sync.dma_start(
                                out=dn_dram[ch2:ch2 + 1, :], in_=dnst
                            )
                            fin_q.append(ch2)
                            del po_tiles[ch2]
                    ch_cur, jt_cur = seq[min(idx, len(seq) - 1)]
                    done = idx >= len(seq)
                    if done:
                        if fin_q:
                            emit_finisher()
                        elif proj_q:
                            emit_proj_one()
                    elif jt_cur == 5 and fin_q:
                        emit_finisher()
                    elif (jt_cur >= 11 or jt_cur <= 2) and proj_q:
                        emit_proj_one()

                while fin_q:
                    emit_finisher()
                while proj_q:
                    emit_proj_one(tail=True)

    nc.finalize()
    return nc


_CACHED = {}


def kernel(x, w_in, w_out, b_out, _trace=False):
    if "nc" not in _CACHED:
        _CACHED["nc"] = _build_nc()
    nc = _CACHED["nc"]

    xf = np.ascontiguousarray(x.reshape(NT, DIM).T.astype(np.float32))
    xTb = np.ascontiguousarray(
        xf.reshape(8, 128, NBLK, 512).transpose(1, 2, 0, 3)
    ).astype(ml_dtypes.bfloat16)

    w_in = np.asarray(w_in, dtype=np.float32)
    w_out = np.asarray(w_out, dtype=np.float32)

    in_maps = []
    for c in range(NCORES):
        h0, h1 = HPC * c, HPC * c + 1
        qcols = list(range(h0 * HD, h0 * HD + HD)) + list(
            range(h1 * HD, h1 * HD + HD)
        )
        kcols = [DIM + q for q in qcols]
        vcols = [2 * DIM + q for q in qcols]
        wqk_c = np.ascontiguousarray(
            w_in[:, qcols + kcols].reshape(8, 128, 2, 128).transpose(1, 0, 2, 3)
        ).astype(ml_dtypes.bfloat16)
        wv_c = np.ascontiguousarray(
            w_in[:, vcols].reshape(8, 128, 128).transpose(1, 0, 2)
        ).astype(ml_dtypes.bfloat16)
        wout_c = np.ascontiguousarray(
            w_out[128 * c:128 * (c + 1), :]
        ).astype(ml_dtypes.bfloat16)
        in_maps.append(
            {"xTb": xTb, "wqk": wqk_c, "wv": wv_c, "wout": wout_c}
        )

    res = run_bass_kernel_spmd(
        nc, in_maps, core_ids=list(range(NCORES)), trace=_trace
    )
    acc = res.results[0]["po"].astype(np.float64)
    for c in range(1, NCORES):
        acc = acc + res.results[c]["po"].astype(np.float64)
    out = acc.T + np.asarray(b_out, dtype=np.float64)
    if _trace:
        kernel.last_result = res
    return np.ascontiguousarray(out.reshape(B, N, DIM).astype(np.float32))
